# revision 14
# baseline (speedup 1.0000x reference)
"""Self-contained Trainium2 Bass kernel for nn_AttentionModel (B=4, S=2048, E=1024).

Model: q/k/v linear projections + scaled-dot-product attention (scale = sqrt(E)).

Sharding (8 NeuronCores): core c handles batch b=c//2, query-row half h=c%2
(1024 q rows). There are NO collectives: both the K and the V paths are
restructured so each core only needs host-provided raw K/V for its batch
plus its own q-half.

  K path:  scores[q,k] = q_proj . (K_raw Wk^T + bk)^T
                       = (q_proj Wk) . K_raw^T + (q_proj . bk)[q]
           The second term is constant per query row, and softmax is
           invariant to adding a per-q constant, so it is dropped exactly.
           The Wk multiply folds into the q side (qw = q_proj @ Wk), which
           costs the same as projecting K (E==E), but needs only the core's
           own 1024 q rows instead of all 2048 k rows -> no k exchange.
  V path:  out[q,f] = sum_e (sum_k attn[q,k] V_raw[k,e]) Wv[f,e]
           (associativity; the Wv multiply moves after the attention
           contraction at identical cost) -> no v exchange.

Total per-core PE work is identical to the classical data-parallel +
pair-exchange decomposition (15.03 GFLOP), but the schedule is pure
feed-forward: the CC cores' ~37us init latency and ~25us/MB AllGather cost
(measured) disappear, along with all staging/readback DMA.

Device algorithm per core (all matmuls bf16 with fp32 PSUM accumulation):
  qT_proj[f,q]  = WqT.T @ qT_in  (+bq via ACT bias on eviction)
  qwT    [e,q]  = Wk.T @ qT_proj           (bk dropped: per-q softmax const)
  scoresT[k,q]  = KrT.T @ qwT              (per 128k x 512q psum tile)
  expT   [k,q]  = exp(scoresT / sqrt(E))   (ACT; logits are O(+-6),
                                            fp32-safe without max-sub)
  avT    [e,q]  = V_rawT.T @ expT          (accumulate over 16 k chunks)
  sums   [q]    = DVE-accumulated exp tiles, partition-folded by a tiny
                  fp32 ones-matmul into per-partition [q,1] layout
  out    [q,f]  = (avT.T @ WvT) * (1/sums) (per-partition ACT scale;
                                            bv added on host)

Schedule notes:
  - The PE p-state ramps with sustained use; warm-up matmuls on a zeroed
    tile run during the framework preamble / first-DMA window so real
    matmuls start at full clock.
  - All input priming rides the sync HWDGE queue in exact consumption
    order with 256-col first chunks; every tensor lands well before its
    consumer (inputs total 16 MB vs 191us of PE work).
  - Output is written as bf16 (error budget allows; host upcasts) per
    512-col half right after each half's eviction; the two halves of each
    q-tile are evicted in parallel on DVE and ACT, so the final drain
    after the last matmul is short.

Host pre-tiles every input into the [128, outer, free] SBUF layout in bf16, so
the device performs no transposes or casts on the inputs.
"""

import sys

for _p in ("/opt/trn_rl_repo", "/root/.axon_site/_ro/trn_rl_repo"):
    if _p not in sys.path:
        sys.path.insert(0, _p)

import numpy as np
import ml_dtypes

import concourse.bacc as bacc
import concourse.mybir as mybir
import concourse.tile as tile
from concourse.bass_utils import run_bass_kernel_spmd

B, S, E = 4, 2048, 1024
P = 128
SQ = S // 2          # q rows per core
N_CORES = 8
EO = E // P          # 8  e-outer chunks
FO = E // P          # 8  f-outer chunks
KC = S // P          # 16 k-row chunks
QB = SQ // 512       # 2  q 512-blocks
INV_SCALE = float(1.0 / np.sqrt(np.float32(E)))

BF16 = mybir.dt.bfloat16
F32 = mybir.dt.float32

_BUILD_CACHE: dict = {}


def _build(with_mask: bool):
    nc = bacc.Bacc(
        "TRN2",
        target_bir_lowering=False,
        debug=False,
        enable_asserts=False,
        num_devices=N_CORES,
    )

    wqk0_d = nc.declare_dram_parameter("wqk0", [P, EO, 256], BF16, isOutput=False)
    wqkr_d = nc.declare_dram_parameter("wqkr", [3, P, EO, 256], BF16, isOutput=False)
    qt0_d = nc.declare_dram_parameter("qt0", [P, EO, 512], BF16, isOutput=False)
    qt_d = nc.declare_dram_parameter("qt", [P, EO, SQ], BF16, isOutput=False)
    krt_d = nc.declare_dram_parameter("krt", [P, EO, S], BF16, isOutput=False)
    vr_d = nc.declare_dram_parameter("vr", [P, 2, KC, 512], BF16, isOutput=False)
    wvt_d = nc.declare_dram_parameter("wvt", [P, EO, E], BF16, isOutput=False)
    bqk_d = nc.declare_dram_parameter("bqk", [P, FO], F32, isOutput=False)
    if with_mask:
        mask_d = nc.declare_dram_parameter("maskt", [P, KC, SQ], BF16, isOutput=False)
    out_d = nc.declare_dram_parameter("out", [P, SQ // P, E], BF16, isOutput=True)

    with tile.TileContext(nc) as tc:
        with (
            tc.tile_pool(name="const", bufs=1) as const,
            tc.tile_pool(name="proj", bufs=1) as proj,
            tc.tile_pool(name="ppsum", bufs=2, space="PSUM") as ppsum,
            tc.tile_pool(name="opsum_a", bufs=2, space="PSUM") as opsum_a,
            tc.tile_pool(name="opsum_b", bufs=2, space="PSUM") as opsum_b,
            tc.tile_pool(name="spsum", bufs=2, space="PSUM") as spsum,
        ):
            # PE warm-up: the tensor engine clock ramps with sustained use
            # (low->mid->full over ~3us of continuous execution). Dummy
            # matmuls on a zeroed tile keep the PE busy from the end of the
            # framework preamble until the first real inputs land (~12us),
            # so real matmuls start at full clock.
            warm_sb = const.tile([P, 256], BF16)
            nc.any.memset(warm_sb[:], 0.0)
            for _ in range(44):
                wp = ppsum.tile([P, 512], F32, tag="pp")
                nc.tensor.matmul(wp[:, 0:256], warm_sb[:, 0:P], warm_sb[:],
                                 start=True, stop=True)

            ones_sb = const.tile([P, 1], F32)
            nc.any.memset(ones_sb[:], 1.0)
            bqk_sb = const.tile([P, FO], F32)
            nc.sync.dma_start(out=bqk_sb[:], in_=bqk_d[:])

            # persistent tensors (live through phase B)
            qwT = proj.tile([P, EO, SQ], BF16)       # [e-inner, eo, q]
            krT = proj.tile([P, EO, S], BF16)        # [e-inner, eo, k]
            v_raw = proj.tile([P, 2, KC, 512], BF16)  # [k-inner, e-half, kc, e]
            wv_sb = proj.tile([P, EO, E], BF16)      # [e-inner, eo, f] = WvT
            avT = proj.tile([P, EO, SQ], BF16)       # [e-inner, eo, q]

            # --- phase A: fused projection qw = query @ (Wq^T Wk) + bqk ---
            with tc.tile_pool(name="io", bufs=1) as io:
                wqk_sb = io.tile([P, EO, E], BF16)
                qt_in = io.tile([P, EO, SQ], BF16)

                # Input priming on the sync HWDGE queue in exact
                # consumption order, pre-chunked contiguous pieces
                # (a dma_start on a busy engine stalls it until the depth-1
                # queue frees, so everything rides the otherwise-idle sync
                # engine).
                nc.sync.dma_start(out=wqk_sb[:, :, 0:256], in_=wqk0_d[:])
                nc.sync.dma_start(out=qt_in[:, :, 0:512], in_=qt0_d[:])
                nc.sync.dma_start(out=wqk_sb[:, :, 256:512], in_=wqkr_d[0])
                nc.sync.dma_start(out=wqk_sb[:, :, 512:768], in_=wqkr_d[1])
                nc.sync.dma_start(out=wqk_sb[:, :, 768:1024], in_=wqkr_d[2])
                nc.sync.dma_start(out=qt_in[:, :, 512:1024],
                                  in_=qt_d[:, :, 512:1024])
                nc.sync.dma_start(out=krT[:, :, 0:S // 2], in_=krt_d[:, :, 0:S // 2])
                nc.sync.dma_start(out=krT[:, :, S // 2:S], in_=krt_d[:, :, S // 2:S])
                nc.sync.dma_start(out=v_raw[:, 0], in_=vr_d[:, 0])
                nc.sync.dma_start(out=v_raw[:, 1], in_=vr_d[:, 1])
                nc.sync.dma_start(out=wv_sb[:], in_=wvt_d[:])

                # fused projection: psum[e128, q512] = sum_ei Wqk.T @ qT
                # qb-outer so the first 8 chains only need qt cols 0:512.
                for qb in range(QB):
                    for eo in range(EO):
                        ps = ppsum.tile([P, 512], F32, tag="pp")
                        for ei in range(EO):
                            nc.tensor.matmul(
                                ps[:],
                                wqk_sb[:, ei, eo * P:(eo + 1) * P],
                                qt_in[:, ei, qb * 512:(qb + 1) * 512],
                                start=(ei == 0),
                                stop=(ei == EO - 1),
                            )
                        nc.scalar.activation(
                            qwT[:, eo, qb * 512:(qb + 1) * 512],
                            ps[:],
                            mybir.ActivationFunctionType.Identity,
                            bias=bqk_sb[:, eo:eo + 1],
                        )

            # ---------------- phase B: attention ----------------
            with (
                tc.tile_pool(name="phb", bufs=2) as phb,
                tc.tile_pool(name="outp", bufs=3) as outp,
                tc.tile_pool(name="rpool", bufs=8) as rpool,
            ):
                if with_mask:
                    mask_sb = phb.tile([P, KC, SQ], BF16, tag="mask", bufs=1)
                    nc.sync.dma_start(out=mask_sb[:], in_=mask_d[:])

                expTs, recips = [], []
                for qb in range(QB):
                    # scoresT[k,q] = KrT.T @ qwT + exp, per 128-k chunk
                    expT = phb.tile([P, KC, 512], BF16, tag="expT")
                    expTs.append(expT)
                    sums_acc = phb.tile([P, 512], F32, tag="sumacc")
                    for kc in range(KC):
                        ps = ppsum.tile([P, 512], F32, tag="pp")
                        for eo in range(EO):
                            nc.tensor.matmul(
                                ps[:],
                                krT[:, eo, kc * P:(kc + 1) * P],
                                qwT[:, eo, qb * 512:(qb + 1) * 512],
                                start=(eo == 0),
                                stop=(eo == EO - 1),
                            )
                        if with_mask:
                            nc.vector.tensor_scalar_mul(ps[:], ps[:], INV_SCALE)
                            nc.vector.tensor_add(
                                ps[:], ps[:],
                                mask_sb[:, kc, qb * 512:(qb + 1) * 512],
                            )
                            nc.scalar.activation(
                                expT[:, kc, :], ps[:],
                                mybir.ActivationFunctionType.Exp,
                            )
                        else:
                            nc.scalar.activation(
                                expT[:, kc, :], ps[:],
                                mybir.ActivationFunctionType.Exp,
                                scale=INV_SCALE,
                            )
                        # accumulate softmax denominators on DVE (frees the
                        # PE from the ones-column matmuls)
                        if kc == 0:
                            nc.vector.tensor_copy(
                                out=sums_acc[:], in_=expT[:, kc, :])
                        else:
                            nc.vector.tensor_add(
                                sums_acc[:], sums_acc[:], expT[:, kc, :])

                    # avT[e,q] = V_rawT.T @ expT, per 128-row e-chunk,
                    # accumulated over the 16 k-chunks
                    for ec in range(EO):
                        pa = opsum_a.tile([P, 512], F32, tag="pa")
                        for kc in range(KC):
                            nc.tensor.matmul(
                                pa[:],
                                v_raw[:, ec // 4, kc,
                                      (ec % 4) * P:(ec % 4 + 1) * P],
                                expT[:, kc, :],
                                start=(kc == 0),
                                stop=(kc == KC - 1),
                            )
                        nc.scalar.activation(
                            avT[:, ec, qb * 512:(qb + 1) * 512], pa[:],
                            mybir.ActivationFunctionType.Identity,
                        )

                    # fold the partition axis with tiny fp32 ones-matmuls:
                    # psum[q128, 1] = sums_acc[:, qslice].T @ ones — lands
                    # directly in the per-partition layout the out-evict
                    # scale needs. Placed after the avT chains so the PE
                    # never waits on the trailing DVE adds.
                    qb_recips = []
                    for qi in range(4):
                        pf = spsum.tile([P, 1], F32, tag="pf")
                        nc.tensor.matmul(
                            pf[:],
                            sums_acc[:, qi * P:(qi + 1) * P],
                            ones_sb[:],
                            start=True, stop=True,
                        )
                        rt = rpool.tile([P, 1], F32, tag="recip")
                        nc.vector.reciprocal(rt[:], pf[:])
                        qb_recips.append(rt)
                    recips.append(qb_recips)

                # output projection: out[q,f] = avT.T @ WvT, scaled by the
                # softmax reciprocal on eviction; each 512-col half is DMA'd
                # as soon as its psum chain is evicted.
                for qb in range(QB):
                    for qi in range(4):
                        qg = qb * 4 + qi
                        rt = recips[qb][qi]
                        pa = opsum_a.tile([P, 512], F32, tag="pa")
                        pb = opsum_b.tile([P, 512], F32, tag="pb")
                        for eo in range(EO):
                            lhsT = avT[:, eo, qg * P:(qg + 1) * P]
                            st, sp = (eo == 0), (eo == EO - 1)
                            nc.tensor.matmul(pa[:], lhsT, wv_sb[:, eo, 0:512],
                                             start=st, stop=sp)
                            nc.tensor.matmul(pb[:], lhsT, wv_sb[:, eo, 512:1024],
                                             start=st, stop=sp)
                        out_sb = outp.tile([P, E], BF16, tag="outsb")
                        nc.vector.tensor_scalar_mul(
                            out_sb[:, 0:512], pa[:], rt[:])
                        nc.sync.dma_start(out=out_d[:, qg, 0:512],
                                          in_=out_sb[:, 0:512])
                        nc.scalar.activation(
                            out_sb[:, 512:1024], pb[:],
                            mybir.ActivationFunctionType.Copy,
                            scale=rt[:],
                        )
                        nc.sync.dma_start(out=out_d[:, qg, 512:1024],
                                          in_=out_sb[:, 512:1024])

    nc.compile()
    return nc


def _bf16_tiled(x):
    """[R, C] fp32 -> [128, R//128, C] bf16 with partition = inner row index."""
    r, c = x.shape
    return (
        np.ascontiguousarray(x).astype(ml_dtypes.bfloat16)
        .reshape(r // P, P, c).transpose(1, 0, 2).copy()
    )


def _prepare_in_maps(query, key, value, attn_mask, Wq, bq, Wk, bk, Wv, bv,
                     with_mask):
    query = np.asarray(query, np.float32)
    key = np.asarray(key, np.float32)
    value = np.asarray(value, np.float32)
    # Constant-fold the two q-side weight matmuls: qw = query @ (Wq^T Wk)
    # + (bq @ Wk); the product is computed once on the host in fp64.
    Wqk = (np.asarray(Wq, np.float64).T @ np.asarray(Wk, np.float64)
           ).astype(np.float32)
    wqkt = _bf16_tiled(Wqk)
    w_t = {
        "wqk0": np.ascontiguousarray(wqkt[:, :, 0:256]),
        "wqkr": np.ascontiguousarray(
            wqkt[:, :, 256:1024].reshape(P, EO, 3, 256).transpose(2, 0, 1, 3)),
        "wvt": _bf16_tiled(np.asarray(Wv, np.float32).T),
    }
    bqk = (np.asarray(bq, np.float64) @ np.asarray(Wk, np.float64)
           ).astype(np.float32)
    bqk_t = bqk.reshape(FO, P).T.copy()
    # NOTE: bk is exactly dropped — it only shifts each query row's logits
    # by a constant, which softmax cancels. bv is added on the host.

    in_maps = []
    for c in range(N_CORES):
        b, h = c // 2, c % 2
        qt = _bf16_tiled(query[b, h * SQ:(h + 1) * SQ, :].T)
        qt0 = np.ascontiguousarray(qt[:, :, 0:512])
        krt = _bf16_tiled(key[b].T)                      # [e-in, eo, 2048 k]
        vr = (
            np.asarray(value[b], np.float32).astype(ml_dtypes.bfloat16)
            .reshape(KC, P, 2, 512).transpose(1, 2, 0, 3).copy()
        )  # [k-in, e-half, kc, e-col]
        m = dict(qt=qt, qt0=qt0, krt=krt, vr=vr, bqk=bqk_t, **w_t)
        if with_mask:
            mt = np.asarray(attn_mask[b, h * SQ:(h + 1) * SQ, :], np.float32).T
            m["maskt"] = (
                mt.astype(ml_dtypes.bfloat16)
                .reshape(KC, P, SQ).transpose(1, 0, 2).copy()
            )
        in_maps.append(m)
    return in_maps


def _run(inputs, trace=False):
    with_mask = bool(np.any(np.asarray(inputs["attn_mask"])))
    key = with_mask
    if key not in _BUILD_CACHE:
        _BUILD_CACHE[key] = _build(with_mask)
    nc = _BUILD_CACHE[key]

    in_maps = _prepare_in_maps(with_mask=with_mask, **inputs)
    res = run_bass_kernel_spmd(nc, in_maps, core_ids=list(range(N_CORES)),
                               trace=trace)

    bv = np.asarray(inputs["bv"], np.float32)
    out = np.zeros((B, S, E), np.float32)
    for c in range(N_CORES):
        b, h = c // 2, c % 2
        oc = np.asarray(res.results[c]["out"], np.float32)  # [P, SQ//P, E]
        out[b, h * SQ:(h + 1) * SQ, :] = (
            oc.transpose(1, 0, 2).reshape(SQ, E) + bv[None, :]
        )
    return out, res


def kernel(**inputs) -> np.ndarray:
    out, _ = _run(inputs, trace=False)
    return out


# revision 16
# speedup vs baseline: 1.2006x; 1.2006x over previous
"""Self-contained Trainium2 Bass kernel for nn_AttentionModel (B=4, S=2048, E=1024).

Model: q/k/v linear projections + scaled-dot-product attention (scale = sqrt(E)).

Sharding (8 NeuronCores): core c handles batch b=c//2, query-row half h=c%2
(1024 q rows). There are NO collectives: both the K and the V paths are
restructured so each core only needs host-provided raw K/V for its batch
plus its own q-half.

  K path:  scores[q,k] = q_proj . (K_raw Wk^T + bk)^T
                       = (q_proj Wk) . K_raw^T + (q_proj . bk)[q]
           The second term is constant per query row, and softmax is
           invariant to adding a per-q constant, so it is dropped exactly.
           The Wk multiply folds into the q side (qw = q_proj @ Wk), which
           costs the same as projecting K (E==E), but needs only the core's
           own 1024 q rows instead of all 2048 k rows -> no k exchange.
  V path:  out[q,f] = sum_e (sum_k attn[q,k] V_raw[k,e]) Wv[f,e]
           (associativity; the Wv multiply moves after the attention
           contraction at identical cost) -> no v exchange.

Total per-core PE work is identical to the classical data-parallel +
pair-exchange decomposition (15.03 GFLOP), but the schedule is pure
feed-forward: the CC cores' ~37us init latency and ~25us/MB AllGather cost
(measured) disappear, along with all staging/readback DMA.

Device algorithm per core (all matmuls bf16 with fp32 PSUM accumulation):
  qT_proj[f,q]  = WqT.T @ qT_in  (+bq via ACT bias on eviction)
  qwT    [e,q]  = Wk.T @ qT_proj           (bk dropped: per-q softmax const)
  scoresT[k,q]  = KrT.T @ qwT              (per 128k x 512q psum tile)
  expT   [k,q]  = exp(scoresT / sqrt(E))   (ACT; logits are O(+-6),
                                            fp32-safe without max-sub)
  avT    [e,q]  = V_rawT.T @ expT          (accumulate over 16 k chunks)
  sums   [q]    = DVE-accumulated exp tiles, partition-folded by a tiny
                  fp32 ones-matmul into per-partition [q,1] layout
  out    [q,f]  = (avT.T @ WvT) * (1/sums) (per-partition ACT scale;
                                            bv added on host)

Schedule notes:
  - The PE p-state ramps with sustained use; warm-up matmuls on a zeroed
    tile run during the framework preamble / first-DMA window so real
    matmuls start at full clock.
  - All input priming rides the sync HWDGE queue in exact consumption
    order with 256-col first chunks; every tensor lands well before its
    consumer (inputs total 16 MB vs 191us of PE work).
  - Output is written as bf16 (error budget allows; host upcasts) per
    512-col half right after each half's eviction; the two halves of each
    q-tile are evicted in parallel on DVE and ACT, so the final drain
    after the last matmul is short.

Host pre-tiles every input into the [128, outer, free] SBUF layout in bf16, so
the device performs no transposes or casts on the inputs.
"""

import sys

for _p in ("/opt/trn_rl_repo", "/root/.axon_site/_ro/trn_rl_repo"):
    if _p not in sys.path:
        sys.path.insert(0, _p)

import numpy as np
import ml_dtypes

import concourse.bacc as bacc
import concourse.mybir as mybir
import concourse.tile as tile
from concourse.bass_utils import run_bass_kernel_spmd

B, S, E = 4, 2048, 1024
P = 128
SQ = S // 2          # q rows per core
N_CORES = 8
EO = E // P          # 8  e-outer chunks
FO = E // P          # 8  f-outer chunks
KC = S // P          # 16 k-row chunks
QB = SQ // 512       # 2  q 512-blocks
INV_SCALE = float(1.0 / np.sqrt(np.float32(E)))

BF16 = mybir.dt.bfloat16
F32 = mybir.dt.float32

_BUILD_CACHE: dict = {}


def _build(with_mask: bool):
    nc = bacc.Bacc(
        "TRN2",
        target_bir_lowering=False,
        debug=False,
        enable_asserts=False,
        num_devices=N_CORES,
    )

    wqk0_d = nc.declare_dram_parameter("wqk0", [P, EO, 256], BF16, isOutput=False)
    wqkr_d = nc.declare_dram_parameter("wqkr", [3, P, EO, 256], BF16, isOutput=False)
    qt0_d = nc.declare_dram_parameter("qt0", [P, EO, 512], BF16, isOutput=False)
    qt_d = nc.declare_dram_parameter("qt", [P, EO, SQ], BF16, isOutput=False)
    krt_d = nc.declare_dram_parameter("krt", [P, EO, S], BF16, isOutput=False)
    vr_d = nc.declare_dram_parameter("vr", [P, 2, KC, 512], BF16, isOutput=False)
    wvt_d = nc.declare_dram_parameter("wvt", [P, EO, E], BF16, isOutput=False)
    bqk_d = nc.declare_dram_parameter("bqk", [P, FO], F32, isOutput=False)
    if with_mask:
        mask_d = nc.declare_dram_parameter("maskt", [P, KC, SQ], BF16, isOutput=False)
    out_d = nc.declare_dram_parameter("out", [P, SQ // P, E], BF16, isOutput=True)

    with tile.TileContext(nc) as tc:
        with (
            tc.tile_pool(name="const", bufs=1) as const,
            tc.tile_pool(name="proj", bufs=1) as proj,
            tc.tile_pool(name="ppsum", bufs=2, space="PSUM") as ppsum,
            tc.tile_pool(name="opsum_a", bufs=2, space="PSUM") as opsum_a,
            tc.tile_pool(name="opsum_b", bufs=2, space="PSUM") as opsum_b,
            tc.tile_pool(name="spsum", bufs=2, space="PSUM") as spsum,
        ):
            # PE warm-up: the tensor engine clock ramps with sustained use
            # (low->mid->full over ~3us of continuous execution). Dummy
            # matmuls on a zeroed tile keep the PE busy from the end of the
            # framework preamble until the first real inputs land (~12us),
            # so real matmuls start at full clock.
            warm_sb = const.tile([P, 256], BF16)
            nc.any.memset(warm_sb[:], 0.0)
            for _ in range(44):
                wp = ppsum.tile([P, 512], F32, tag="pp")
                nc.tensor.matmul(wp[:, 0:256], warm_sb[:, 0:P], warm_sb[:],
                                 start=True, stop=True)

            ones_sb = const.tile([P, 1], F32)
            nc.any.memset(ones_sb[:], 1.0)
            bqk_sb = const.tile([P, FO], F32)
            nc.sync.dma_start(out=bqk_sb[:], in_=bqk_d[:])

            # persistent tensors (live through phase B)
            qwT = proj.tile([P, EO, SQ], BF16)       # [e-inner, eo, q]
            krT = proj.tile([P, EO, S], BF16)        # [e-inner, eo, k]
            v_raw = proj.tile([P, 2, KC, 512], BF16)  # [k-inner, e-half, kc, e]
            wv_sb = proj.tile([P, EO, E], BF16)      # [e-inner, eo, f] = WvT
            avT = proj.tile([P, EO, SQ], BF16)       # [e-inner, eo, q]

            # --- phase A: fused projection qw = query @ (Wq^T Wk) + bqk ---
            with tc.tile_pool(name="io", bufs=1) as io:
                wqk_sb = io.tile([P, EO, E], BF16)
                qt_in = io.tile([P, EO, SQ], BF16)

                # Input priming on the sync HWDGE queue in exact
                # consumption order, pre-chunked contiguous pieces
                # (a dma_start on a busy engine stalls it until the depth-1
                # queue frees, so everything rides the otherwise-idle sync
                # engine).
                nc.sync.dma_start(out=wqk_sb[:, :, 0:256], in_=wqk0_d[:])
                nc.sync.dma_start(out=qt_in[:, :, 0:512], in_=qt0_d[:])
                nc.sync.dma_start(out=wqk_sb[:, :, 256:512], in_=wqkr_d[0])
                nc.sync.dma_start(out=wqk_sb[:, :, 512:768], in_=wqkr_d[1])
                nc.sync.dma_start(out=wqk_sb[:, :, 768:1024], in_=wqkr_d[2])
                nc.sync.dma_start(out=qt_in[:, :, 512:1024],
                                  in_=qt_d[:, :, 512:1024])
                nc.sync.dma_start(out=krT[:, :, 0:S // 2], in_=krt_d[:, :, 0:S // 2])
                nc.sync.dma_start(out=krT[:, :, S // 2:S], in_=krt_d[:, :, S // 2:S])
                nc.sync.dma_start(out=v_raw[:, 0], in_=vr_d[:, 0])
                nc.sync.dma_start(out=v_raw[:, 1], in_=vr_d[:, 1])
                nc.sync.dma_start(out=wv_sb[:], in_=wvt_d[:])

                # fused projection: psum[e128, q512] = sum_ei Wqk.T @ qT
                # qb-outer so the first 8 chains only need qt cols 0:512.
                for qb in range(QB):
                    for eo in range(EO):
                        ps = ppsum.tile([P, 512], F32, tag="pp")
                        for ei in range(EO):
                            nc.tensor.matmul(
                                ps[:],
                                wqk_sb[:, ei, eo * P:(eo + 1) * P],
                                qt_in[:, ei, qb * 512:(qb + 1) * 512],
                                start=(ei == 0),
                                stop=(ei == EO - 1),
                            )
                        nc.scalar.activation(
                            qwT[:, eo, qb * 512:(qb + 1) * 512],
                            ps[:],
                            mybir.ActivationFunctionType.Identity,
                            bias=bqk_sb[:, eo:eo + 1],
                        )

            # ---------------- phase B: attention ----------------
            with (
                tc.tile_pool(name="phb", bufs=2) as phb,
                tc.tile_pool(name="outp", bufs=3) as outp,
                tc.tile_pool(name="rpool", bufs=8) as rpool,
            ):
                if with_mask:
                    mask_sb = phb.tile([P, KC, SQ], BF16, tag="mask", bufs=1)
                    nc.sync.dma_start(out=mask_sb[:], in_=mask_d[:])

                expTs, recips = [], []
                for qb in range(QB):
                    # scoresT[k,q] = KrT.T @ qwT + exp, per 128-k chunk
                    expT = phb.tile([P, KC, 512], BF16, tag="expT")
                    expTs.append(expT)
                    sums_acc = phb.tile([P, 512], F32, tag="sumacc")
                    for kc in range(KC):
                        ps = ppsum.tile([P, 512], F32, tag="pp")
                        for eo in range(EO):
                            nc.tensor.matmul(
                                ps[:],
                                krT[:, eo, kc * P:(kc + 1) * P],
                                qwT[:, eo, qb * 512:(qb + 1) * 512],
                                start=(eo == 0),
                                stop=(eo == EO - 1),
                            )
                        if with_mask:
                            nc.vector.tensor_scalar_mul(ps[:], ps[:], INV_SCALE)
                            nc.vector.tensor_add(
                                ps[:], ps[:],
                                mask_sb[:, kc, qb * 512:(qb + 1) * 512],
                            )
                            nc.scalar.activation(
                                expT[:, kc, :], ps[:],
                                mybir.ActivationFunctionType.Exp,
                            )
                        else:
                            nc.scalar.activation(
                                expT[:, kc, :], ps[:],
                                mybir.ActivationFunctionType.Exp,
                                scale=INV_SCALE,
                            )
                        # accumulate softmax denominators on DVE (frees the
                        # PE from the ones-column matmuls)
                        if kc == 0:
                            nc.vector.tensor_copy(
                                out=sums_acc[:], in_=expT[:, kc, :])
                        else:
                            nc.vector.tensor_add(
                                sums_acc[:], sums_acc[:], expT[:, kc, :])

                    # avT[e,q] = V_rawT.T @ expT, per 128-row e-chunk,
                    # accumulated over the 16 k-chunks
                    for ec in range(EO):
                        pa = opsum_a.tile([P, 512], F32, tag="pa")
                        for kc in range(KC):
                            nc.tensor.matmul(
                                pa[:],
                                v_raw[:, ec // 4, kc,
                                      (ec % 4) * P:(ec % 4 + 1) * P],
                                expT[:, kc, :],
                                start=(kc == 0),
                                stop=(kc == KC - 1),
                            )
                        nc.scalar.activation(
                            avT[:, ec, qb * 512:(qb + 1) * 512], pa[:],
                            mybir.ActivationFunctionType.Identity,
                        )

                    # fold the partition axis with tiny fp32 ones-matmuls:
                    # psum[q128, 1] = sums_acc[:, qslice].T @ ones — lands
                    # directly in the per-partition layout the out-evict
                    # scale needs. Placed after the avT chains so the PE
                    # never waits on the trailing DVE adds.
                    qb_recips = []
                    for qi in range(4):
                        pf = spsum.tile([P, 1], F32, tag="pf")
                        nc.tensor.matmul(
                            pf[:],
                            sums_acc[:, qi * P:(qi + 1) * P],
                            ones_sb[:],
                            start=True, stop=True,
                        )
                        rt = rpool.tile([P, 1], F32, tag="recip")
                        nc.vector.reciprocal(rt[:], pf[:])
                        qb_recips.append(rt)
                    recips.append(qb_recips)

                # output projection: out[q,f] = avT.T @ WvT, scaled by the
                # softmax reciprocal on eviction; each 512-col half is DMA'd
                # as soon as its psum chain is evicted.
                for qb in range(QB):
                    for qi in range(4):
                        qg = qb * 4 + qi
                        rt = recips[qb][qi]
                        pa = opsum_a.tile([P, 512], F32, tag="pa")
                        pb = opsum_b.tile([P, 512], F32, tag="pb")
                        for eo in range(EO):
                            lhsT = avT[:, eo, qg * P:(qg + 1) * P]
                            st, sp = (eo == 0), (eo == EO - 1)
                            nc.tensor.matmul(pa[:], lhsT, wv_sb[:, eo, 0:512],
                                             start=st, stop=sp)
                            nc.tensor.matmul(pb[:], lhsT, wv_sb[:, eo, 512:1024],
                                             start=st, stop=sp)
                        out_sb = outp.tile([P, E], BF16, tag="outsb")
                        nc.vector.tensor_scalar_mul(
                            out_sb[:, 0:512], pa[:], rt[:])
                        nc.sync.dma_start(out=out_d[:, qg, 0:512],
                                          in_=out_sb[:, 0:512])
                        nc.scalar.activation(
                            out_sb[:, 512:1024], pb[:],
                            mybir.ActivationFunctionType.Copy,
                            scale=rt[:],
                        )
                        nc.sync.dma_start(out=out_d[:, qg, 512:1024],
                                          in_=out_sb[:, 512:1024])

    nc.compile()
    return nc


def _bf16_tiled(x):
    """[R, C] fp32 -> [128, R//128, C] bf16 with partition = inner row index."""
    r, c = x.shape
    return (
        np.ascontiguousarray(x).astype(ml_dtypes.bfloat16)
        .reshape(r // P, P, c).transpose(1, 0, 2).copy()
    )


def _prepare_in_maps(query, key, value, attn_mask, Wq, bq, Wk, bk, Wv, bv,
                     with_mask):
    query = np.asarray(query, np.float32)
    key = np.asarray(key, np.float32)
    value = np.asarray(value, np.float32)
    # Constant-fold the two q-side weight matmuls: qw = query @ (Wq^T Wk)
    # + (bq @ Wk); the product is computed once on the host in fp64.
    Wqk = (np.asarray(Wq, np.float64).T @ np.asarray(Wk, np.float64)
           ).astype(np.float32)
    wqkt = _bf16_tiled(Wqk)
    w_t = {
        "wqk0": np.ascontiguousarray(wqkt[:, :, 0:256]),
        "wqkr": np.ascontiguousarray(
            wqkt[:, :, 256:1024].reshape(P, EO, 3, 256).transpose(2, 0, 1, 3)),
        "wvt": _bf16_tiled(np.asarray(Wv, np.float32).T),
    }
    bqk = (np.asarray(bq, np.float64) @ np.asarray(Wk, np.float64)
           ).astype(np.float32)
    bqk_t = bqk.reshape(FO, P).T.copy()
    # NOTE: bk is exactly dropped — it only shifts each query row's logits
    # by a constant, which softmax cancels. bv is added on the host.

    in_maps = []
    for c in range(N_CORES):
        b, h = c // 2, c % 2
        qt = _bf16_tiled(query[b, h * SQ:(h + 1) * SQ, :].T)
        qt0 = np.ascontiguousarray(qt[:, :, 0:512])
        krt = _bf16_tiled(key[b].T)                      # [e-in, eo, 2048 k]
        vr = (
            np.asarray(value[b], np.float32).astype(ml_dtypes.bfloat16)
            .reshape(KC, P, 2, 512).transpose(1, 2, 0, 3).copy()
        )  # [k-in, e-half, kc, e-col]
        m = dict(qt=qt, qt0=qt0, krt=krt, vr=vr, bqk=bqk_t, **w_t)
        if with_mask:
            mt = np.asarray(attn_mask[b, h * SQ:(h + 1) * SQ, :], np.float32).T
            m["maskt"] = (
                mt.astype(ml_dtypes.bfloat16)
                .reshape(KC, P, SQ).transpose(1, 0, 2).copy()
            )
        in_maps.append(m)
    return in_maps


def _run(inputs, trace=False):
    with_mask = bool(np.any(np.asarray(inputs["attn_mask"])))
    key = with_mask
    if key not in _BUILD_CACHE:
        _BUILD_CACHE[key] = _build(with_mask)
    nc = _BUILD_CACHE[key]

    in_maps = _prepare_in_maps(with_mask=with_mask, **inputs)
    res = run_bass_kernel_spmd(nc, in_maps, core_ids=list(range(N_CORES)),
                               trace=trace)

    bv = np.asarray(inputs["bv"], np.float32)
    out = np.zeros((B, S, E), np.float32)
    for c in range(N_CORES):
        b, h = c // 2, c % 2
        oc = np.asarray(res.results[c]["out"], np.float32)  # [P, SQ//P, E]
        out[b, h * SQ:(h + 1) * SQ, :] = (
            oc.transpose(1, 0, 2).reshape(SQ, E) + bv[None, :]
        )
    return out, res


def kernel(**inputs) -> np.ndarray:
    out, _ = _run(inputs, trace=False)
    return out


# revision 17
# speedup vs baseline: 1.2028x; 1.0019x over previous
"""Self-contained Trainium2 Bass kernel for nn_AttentionModel (B=4, S=2048, E=1024).

Model: q/k/v linear projections + scaled-dot-product attention (scale = sqrt(E)).

Sharding (8 NeuronCores): core c handles batch b=c//2, query-row half h=c%2
(1024 q rows). There are NO collectives: both the K and the V paths are
restructured so each core only needs host-provided raw K/V for its batch
plus its own q-half.

  K path:  scores[q,k] = q_proj . (K_raw Wk^T + bk)^T
                       = (q_proj Wk) . K_raw^T + (q_proj . bk)[q]
           The second term is constant per query row, and softmax is
           invariant to adding a per-q constant, so it is dropped exactly.
           The Wk multiply folds into the q side (qw = q_proj @ Wk), which
           costs the same as projecting K (E==E), but needs only the core's
           own 1024 q rows instead of all 2048 k rows -> no k exchange.
  V path:  out[q,f] = sum_e (sum_k attn[q,k] V_raw[k,e]) Wv[f,e]
           (associativity; the Wv multiply moves after the attention
           contraction at identical cost) -> no v exchange.

Total per-core PE work is identical to the classical data-parallel +
pair-exchange decomposition (15.03 GFLOP), but the schedule is pure
feed-forward: the CC cores' ~37us init latency and ~25us/MB AllGather cost
(measured) disappear, along with all staging/readback DMA.

Device algorithm per core (all matmuls bf16 with fp32 PSUM accumulation):
  qT_proj[f,q]  = WqT.T @ qT_in  (+bq via ACT bias on eviction)
  qwT    [e,q]  = Wk.T @ qT_proj           (bk dropped: per-q softmax const)
  scoresT[k,q]  = KrT.T @ qwT              (per 128k x 512q psum tile)
  expT   [k,q]  = exp(scoresT / sqrt(E))   (ACT; logits are O(+-6),
                                            fp32-safe without max-sub)
  avT    [e,q]  = V_rawT.T @ expT          (accumulate over 16 k chunks)
  sums   [q]    = DVE-accumulated exp tiles, partition-folded by a tiny
                  fp32 ones-matmul into per-partition [q,1] layout
  out    [q,f]  = (avT.T @ WvT) * (1/sums) (per-partition ACT scale;
                                            bv added on host)

Schedule notes:
  - The PE p-state ramps with sustained use; warm-up matmuls on a zeroed
    tile run during the framework preamble / first-DMA window so real
    matmuls start at full clock.
  - All input priming rides the sync HWDGE queue in exact consumption
    order with 256-col first chunks; every tensor lands well before its
    consumer (inputs total 16 MB vs 191us of PE work).
  - Output is written as bf16 (error budget allows; host upcasts) per
    512-col half right after each half's eviction; the two halves of each
    q-tile are evicted in parallel on DVE and ACT, so the final drain
    after the last matmul is short.

Host pre-tiles every input into the [128, outer, free] SBUF layout in bf16, so
the device performs no transposes or casts on the inputs.
"""

import sys

for _p in ("/opt/trn_rl_repo", "/root/.axon_site/_ro/trn_rl_repo"):
    if _p not in sys.path:
        sys.path.insert(0, _p)

import numpy as np
import ml_dtypes

import concourse.bacc as bacc
import concourse.mybir as mybir
import concourse.tile as tile
from concourse.bass_utils import run_bass_kernel_spmd

B, S, E = 4, 2048, 1024
P = 128
SQ = S // 2          # q rows per core
N_CORES = 8
EO = E // P          # 8  e-outer chunks
FO = E // P          # 8  f-outer chunks
KC = S // P          # 16 k-row chunks
QB = SQ // 512       # 2  q 512-blocks
INV_SCALE = float(1.0 / np.sqrt(np.float32(E)))

BF16 = mybir.dt.bfloat16
F32 = mybir.dt.float32

_BUILD_CACHE: dict = {}


def _build(with_mask: bool):
    nc = bacc.Bacc(
        "TRN2",
        target_bir_lowering=False,
        debug=False,
        enable_asserts=False,
        num_devices=N_CORES,
    )

    wqk0_d = nc.declare_dram_parameter("wqk0", [P, EO, 256], BF16, isOutput=False)
    wqkr_d = nc.declare_dram_parameter("wqkr", [3, P, EO, 256], BF16, isOutput=False)
    qt0_d = nc.declare_dram_parameter("qt0", [P, EO, 512], BF16, isOutput=False)
    qt_d = nc.declare_dram_parameter("qt", [P, EO, SQ], BF16, isOutput=False)
    krt_d = nc.declare_dram_parameter("krt", [P, EO, S], BF16, isOutput=False)
    vr_d = nc.declare_dram_parameter("vr", [P, 2, KC, 512], BF16, isOutput=False)
    wvt_d = nc.declare_dram_parameter("wvt", [P, EO, E], BF16, isOutput=False)
    bqk_d = nc.declare_dram_parameter("bqk", [P, FO], F32, isOutput=False)
    if with_mask:
        mask_d = nc.declare_dram_parameter("maskt", [P, KC, SQ], BF16, isOutput=False)
    out_d = nc.declare_dram_parameter("out", [P, SQ // P, E], BF16, isOutput=True)

    with tile.TileContext(nc) as tc:
        with (
            tc.tile_pool(name="const", bufs=1) as const,
            tc.tile_pool(name="proj", bufs=1) as proj,
            tc.tile_pool(name="ppsum", bufs=2, space="PSUM") as ppsum,
            tc.tile_pool(name="opsum_a", bufs=2, space="PSUM") as opsum_a,
            tc.tile_pool(name="opsum_b", bufs=2, space="PSUM") as opsum_b,
            tc.tile_pool(name="spsum", bufs=2, space="PSUM") as spsum,
        ):
            # PE warm-up: the tensor engine clock ramps with sustained use
            # (low->mid->full over ~3us of continuous execution). Dummy
            # matmuls on a zeroed tile keep the PE busy from the end of the
            # framework preamble until the first real inputs land (~12us),
            # so real matmuls start at full clock.
            warm_sb = const.tile([P, 256], BF16)
            nc.any.memset(warm_sb[:], 0.0)
            for _ in range(44):
                wp = ppsum.tile([P, 512], F32, tag="pp")
                nc.tensor.matmul(wp[:, 0:256], warm_sb[:, 0:P], warm_sb[:],
                                 start=True, stop=True)

            ones_sb = const.tile([P, 1], F32)
            nc.any.memset(ones_sb[:], 1.0)
            bqk_sb = const.tile([P, FO], F32)
            nc.sync.dma_start(out=bqk_sb[:], in_=bqk_d[:])

            # persistent tensors (live through phase B)
            qwT = proj.tile([P, EO, SQ], BF16)       # [e-inner, eo, q]
            krT = proj.tile([P, EO, S], BF16)        # [e-inner, eo, k]
            v_raw = proj.tile([P, 2, KC, 512], BF16)  # [k-inner, e-half, kc, e]
            wv_sb = proj.tile([P, EO, E], BF16)      # [e-inner, eo, f] = WvT
            avT = proj.tile([P, EO, SQ], BF16)       # [e-inner, eo, q]

            # --- phase A: fused projection qw = query @ (Wq^T Wk) + bqk ---
            with tc.tile_pool(name="io", bufs=1) as io:
                wqk_sb = io.tile([P, EO, E], BF16)
                qt_in = io.tile([P, EO, SQ], BF16)

                # Input priming on the sync HWDGE queue in exact
                # consumption order, pre-chunked contiguous pieces
                # (a dma_start on a busy engine stalls it until the depth-1
                # queue frees, so everything rides the otherwise-idle sync
                # engine).
                nc.sync.dma_start(out=wqk_sb[:, :, 0:256], in_=wqk0_d[:])
                nc.sync.dma_start(out=qt_in[:, :, 0:512], in_=qt0_d[:])
                nc.sync.dma_start(out=wqk_sb[:, :, 256:512], in_=wqkr_d[0])
                nc.sync.dma_start(out=wqk_sb[:, :, 512:768], in_=wqkr_d[1])
                nc.sync.dma_start(out=wqk_sb[:, :, 768:1024], in_=wqkr_d[2])
                nc.sync.dma_start(out=qt_in[:, :, 512:1024],
                                  in_=qt_d[:, :, 512:1024])
                nc.sync.dma_start(out=krT[:, :, 0:S // 2], in_=krt_d[:, :, 0:S // 2])
                nc.sync.dma_start(out=krT[:, :, S // 2:S], in_=krt_d[:, :, S // 2:S])
                nc.sync.dma_start(out=v_raw[:, 0], in_=vr_d[:, 0])
                nc.sync.dma_start(out=v_raw[:, 1], in_=vr_d[:, 1])
                nc.sync.dma_start(out=wv_sb[:], in_=wvt_d[:])

                # fused projection: psum[e128, q512] = sum_ei Wqk.T @ qT
                # qb-outer so the first 8 chains only need qt cols 0:512.
                for qb in range(QB):
                    for eo in range(EO):
                        ps = ppsum.tile([P, 512], F32, tag="pp")
                        for ei in range(EO):
                            nc.tensor.matmul(
                                ps[:],
                                wqk_sb[:, ei, eo * P:(eo + 1) * P],
                                qt_in[:, ei, qb * 512:(qb + 1) * 512],
                                start=(ei == 0),
                                stop=(ei == EO - 1),
                            )
                        nc.scalar.activation(
                            qwT[:, eo, qb * 512:(qb + 1) * 512],
                            ps[:],
                            mybir.ActivationFunctionType.Identity,
                            bias=bqk_sb[:, eo:eo + 1],
                        )

            # ---------------- phase B: attention ----------------
            with (
                tc.tile_pool(name="phb", bufs=2) as phb,
                tc.tile_pool(name="outp", bufs=3) as outp,
                tc.tile_pool(name="rpool", bufs=8) as rpool,
            ):
                if with_mask:
                    mask_sb = phb.tile([P, KC, SQ], BF16, tag="mask", bufs=1)
                    nc.sync.dma_start(out=mask_sb[:], in_=mask_d[:])

                expTs, recips = [], []
                for qb in range(QB):
                    # scoresT[k,q] = KrT.T @ qwT + exp, per 128-k chunk
                    expT = phb.tile([P, KC, 512], BF16, tag="expT")
                    expTs.append(expT)
                    sums_acc = phb.tile([P, 512], F32, tag="sumacc")
                    for kc in range(KC):
                        ps = ppsum.tile([P, 512], F32, tag="pp")
                        for eo in range(EO):
                            nc.tensor.matmul(
                                ps[:],
                                krT[:, eo, kc * P:(kc + 1) * P],
                                qwT[:, eo, qb * 512:(qb + 1) * 512],
                                start=(eo == 0),
                                stop=(eo == EO - 1),
                            )
                        if with_mask:
                            nc.vector.tensor_scalar_mul(ps[:], ps[:], INV_SCALE)
                            nc.vector.tensor_add(
                                ps[:], ps[:],
                                mask_sb[:, kc, qb * 512:(qb + 1) * 512],
                            )
                            nc.scalar.activation(
                                expT[:, kc, :], ps[:],
                                mybir.ActivationFunctionType.Exp,
                            )
                        else:
                            nc.scalar.activation(
                                expT[:, kc, :], ps[:],
                                mybir.ActivationFunctionType.Exp,
                                scale=INV_SCALE,
                            )
                        # accumulate softmax denominators on DVE (frees the
                        # PE from the ones-column matmuls)
                        if kc == 0:
                            nc.vector.tensor_copy(
                                out=sums_acc[:], in_=expT[:, kc, :])
                        else:
                            nc.vector.tensor_add(
                                sums_acc[:], sums_acc[:], expT[:, kc, :])

                    # avT[e,q] = V_rawT.T @ expT, per 128-row e-chunk,
                    # accumulated over the 16 k-chunks
                    for ec in range(EO):
                        pa = opsum_a.tile([P, 512], F32, tag="pa")
                        for kc in range(KC):
                            nc.tensor.matmul(
                                pa[:],
                                v_raw[:, ec // 4, kc,
                                      (ec % 4) * P:(ec % 4 + 1) * P],
                                expT[:, kc, :],
                                start=(kc == 0),
                                stop=(kc == KC - 1),
                            )
                        nc.scalar.activation(
                            avT[:, ec, qb * 512:(qb + 1) * 512], pa[:],
                            mybir.ActivationFunctionType.Identity,
                        )

                    # fold the partition axis with tiny fp32 ones-matmuls:
                    # psum[q128, 1] = sums_acc[:, qslice].T @ ones — lands
                    # directly in the per-partition layout the out-evict
                    # scale needs. Placed after the avT chains so the PE
                    # never waits on the trailing DVE adds.
                    qb_recips = []
                    for qi in range(4):
                        pf = spsum.tile([P, 1], F32, tag="pf")
                        nc.tensor.matmul(
                            pf[:],
                            sums_acc[:, qi * P:(qi + 1) * P],
                            ones_sb[:],
                            start=True, stop=True,
                        )
                        rt = rpool.tile([P, 1], F32, tag="recip")
                        nc.vector.reciprocal(rt[:], pf[:])
                        qb_recips.append(rt)
                    recips.append(qb_recips)

                # output projection: out[q,f] = avT.T @ WvT, scaled by the
                # softmax reciprocal on eviction; each 512-col half is DMA'd
                # as soon as its psum chain is evicted.
                for qb in range(QB):
                    for qi in range(4):
                        qg = qb * 4 + qi
                        rt = recips[qb][qi]
                        pa = opsum_a.tile([P, 512], F32, tag="pa")
                        pb = opsum_b.tile([P, 512], F32, tag="pb")
                        for eo in range(EO):
                            lhsT = avT[:, eo, qg * P:(qg + 1) * P]
                            st, sp = (eo == 0), (eo == EO - 1)
                            nc.tensor.matmul(pa[:], lhsT, wv_sb[:, eo, 0:512],
                                             start=st, stop=sp)
                            nc.tensor.matmul(pb[:], lhsT, wv_sb[:, eo, 512:1024],
                                             start=st, stop=sp)
                        out_sb = outp.tile([P, E], BF16, tag="outsb")
                        nc.vector.tensor_scalar_mul(
                            out_sb[:, 0:512], pa[:], rt[:])
                        nc.sync.dma_start(out=out_d[:, qg, 0:512],
                                          in_=out_sb[:, 0:512])
                        nc.scalar.activation(
                            out_sb[:, 512:1024], pb[:],
                            mybir.ActivationFunctionType.Copy,
                            scale=rt[:],
                        )
                        if qg == 2 * QB * 4 // 2 - 1:
                            # last tile: pb half rides the empty act queue as
                            # the ACT engine's final instruction, so the two
                            # final output transfers drain in parallel
                            nc.scalar.dma_start(out=out_d[:, qg, 512:1024],
                                                in_=out_sb[:, 512:1024])
                        else:
                            nc.sync.dma_start(out=out_d[:, qg, 512:1024],
                                              in_=out_sb[:, 512:1024])

    nc.compile()
    return nc


def _bf16_tiled(x):
    """[R, C] fp32 -> [128, R//128, C] bf16 with partition = inner row index."""
    r, c = x.shape
    return (
        np.ascontiguousarray(x).astype(ml_dtypes.bfloat16)
        .reshape(r // P, P, c).transpose(1, 0, 2).copy()
    )


def _prepare_in_maps(query, key, value, attn_mask, Wq, bq, Wk, bk, Wv, bv,
                     with_mask):
    query = np.asarray(query, np.float32)
    key = np.asarray(key, np.float32)
    value = np.asarray(value, np.float32)
    # Constant-fold the two q-side weight matmuls: qw = query @ (Wq^T Wk)
    # + (bq @ Wk); the product is computed once on the host in fp64.
    Wqk = (np.asarray(Wq, np.float64).T @ np.asarray(Wk, np.float64)
           ).astype(np.float32)
    wqkt = _bf16_tiled(Wqk)
    w_t = {
        "wqk0": np.ascontiguousarray(wqkt[:, :, 0:256]),
        "wqkr": np.ascontiguousarray(
            wqkt[:, :, 256:1024].reshape(P, EO, 3, 256).transpose(2, 0, 1, 3)),
        "wvt": _bf16_tiled(np.asarray(Wv, np.float32).T),
    }
    bqk = (np.asarray(bq, np.float64) @ np.asarray(Wk, np.float64)
           ).astype(np.float32)
    bqk_t = bqk.reshape(FO, P).T.copy()
    # NOTE: bk is exactly dropped — it only shifts each query row's logits
    # by a constant, which softmax cancels. bv is added on the host.

    in_maps = []
    for c in range(N_CORES):
        b, h = c // 2, c % 2
        qt = _bf16_tiled(query[b, h * SQ:(h + 1) * SQ, :].T)
        qt0 = np.ascontiguousarray(qt[:, :, 0:512])
        krt = _bf16_tiled(key[b].T)                      # [e-in, eo, 2048 k]
        vr = (
            np.asarray(value[b], np.float32).astype(ml_dtypes.bfloat16)
            .reshape(KC, P, 2, 512).transpose(1, 2, 0, 3).copy()
        )  # [k-in, e-half, kc, e-col]
        m = dict(qt=qt, qt0=qt0, krt=krt, vr=vr, bqk=bqk_t, **w_t)
        if with_mask:
            mt = np.asarray(attn_mask[b, h * SQ:(h + 1) * SQ, :], np.float32).T
            m["maskt"] = (
                mt.astype(ml_dtypes.bfloat16)
                .reshape(KC, P, SQ).transpose(1, 0, 2).copy()
            )
        in_maps.append(m)
    return in_maps


def _run(inputs, trace=False):
    with_mask = bool(np.any(np.asarray(inputs["attn_mask"])))
    key = with_mask
    if key not in _BUILD_CACHE:
        _BUILD_CACHE[key] = _build(with_mask)
    nc = _BUILD_CACHE[key]

    in_maps = _prepare_in_maps(with_mask=with_mask, **inputs)
    res = run_bass_kernel_spmd(nc, in_maps, core_ids=list(range(N_CORES)),
                               trace=trace)

    bv = np.asarray(inputs["bv"], np.float32)
    out = np.zeros((B, S, E), np.float32)
    for c in range(N_CORES):
        b, h = c // 2, c % 2
        oc = np.asarray(res.results[c]["out"], np.float32)  # [P, SQ//P, E]
        out[b, h * SQ:(h + 1) * SQ, :] = (
            oc.transpose(1, 0, 2).reshape(SQ, E) + bv[None, :]
        )
    return out, res


def kernel(**inputs) -> np.ndarray:
    out, _ = _run(inputs, trace=False)
    return out
